# revision 44
# baseline (speedup 1.0000x reference)
"""Trainium2 Bass kernel for nn_EncoderLayer (S=2048, B=4, E=768, F=3072, H=12).

Sharding: 8 cores, core c = 2*b + j handles batch b (b=c//2) with heads
j*6..j*6+5 (tensor-parallel attention).  After out_proj a pairwise
ReduceScatter leaves core 2b+j with SH=1024 sequence rows of batch b.

All heavy matmuls run as fp8e4 DoubleRow (contraction folded 128x2).  Host
pre-scales weights by powers of two so fp8 stays in its normal range; inverse
scales fold into activation scale factors.  fc1/fc2 use two-term weight
error-feedback (W ~ a + b, both fp8) to cancel weight-quantization error.

exp(scores) splits between ScalarE (native Exp -> fp8) and DVE (Schraudolph:
fp8e4 bits = int8(s*8*log2e + 55.55), one pass, bitcast to fp8).  Softmax
denominators come from tiny DR matmuls (ex^T @ ones -> [128 q, 1]) batched
into one reciprocal per head-pair; LN1's rstd uses exp(-ln(var)/2) to stay
inside the exp activation table (no table thrash mid-attention).
"""

from contextlib import ExitStack

import numpy as np
import ml_dtypes

import concourse.bass as bass
import concourse.tile as tile
from concourse import bacc, mybir
from concourse.bass_utils import run_bass_kernel_spmd
from concourse.masks import make_identity

F32 = mybir.dt.float32
BF16 = mybir.dt.bfloat16
FP8 = mybir.dt.float8e4
I8 = mybir.dt.int8
NPBF = ml_dtypes.bfloat16
NPF8 = ml_dtypes.float8_e4m3
AOP = mybir.AluOpType
ACT = mybir.ActivationFunctionType
PM = mybir.MatmulPerfMode

S, B, E, FF = 2048, 4, 768, 3072
H, DH = 12, 64
NCORES = 8
HPC = 6
EO = HPC * DH           # 384
EOP = 512               # padded q/k cols & out_proj rows
SH = S // 2
KC = E // 128
EG = KC // 2            # 3 DR groups over E
FG = FF // 256          # 12 DR groups over F
TBF = S // 128
TBH = SH // 128
NC = 8                  # attention q-blocks
QB = S // NC            # 256
EPS = 1e-5

QK_SC = 64.0
SC_SCALE = 1.0 / 4096.0
V_SC = 16.0
WO_SC = 16.0
OP_SCALE = 1.0 / 1024.0   # 64 (ao) * 16 (wo)
W12_SC = 64.0
S64 = 1.0 / 64.0

SCHRAUD_A = 8.0 / float(np.log(2.0)) * SC_SCALE
SCHRAUD_B = 55.55

REPLICA_GROUPS = [[0, 1], [2, 3], [4, 5], [6, 7]]


def build_program(flags, for_sim=False):
    nc = bacc.Bacc(None, target_bir_lowering=False)

    x8 = nc.dram_tensor("x8", [E, S], FP8, kind="ExternalInput")
    xres = nc.dram_tensor("xres", [SH, E], BF16, kind="ExternalInput")
    wq = nc.dram_tensor("wq", [E, EOP], FP8, kind="ExternalInput")
    wk = nc.dram_tensor("wk", [E, EOP], FP8, kind="ExternalInput")
    wv = nc.dram_tensor("wv", [E, EO], FP8, kind="ExternalInput")
    wo = nc.dram_tensor("wo", [EOP, E], FP8, kind="ExternalInput")
    w1 = nc.dram_tensor("w1", [2, E, FF], FP8, kind="ExternalInput")
    w2 = nc.dram_tensor("w2", [2, FF, E], FP8, kind="ExternalInput")
    bq = nc.dram_tensor("bq", [EOP], F32, kind="ExternalInput")
    bk = nc.dram_tensor("bk", [EOP], F32, kind="ExternalInput")
    bv = nc.dram_tensor("bv", [EO], F32, kind="ExternalInput")
    bo = nc.dram_tensor("bo", [E], F32, kind="ExternalInput")
    b1 = nc.dram_tensor("b1", [FF], F32, kind="ExternalInput")
    b2 = nc.dram_tensor("b2", [E], F32, kind="ExternalInput")
    g1 = nc.dram_tensor("g1", [E], F32, kind="ExternalInput")
    be1 = nc.dram_tensor("be1", [E], F32, kind="ExternalInput")
    g2 = nc.dram_tensor("g2", [E], F32, kind="ExternalInput")
    be2 = nc.dram_tensor("be2", [E], F32, kind="ExternalInput")
    y = nc.dram_tensor("y", [SH, E], F32, kind="ExternalOutput")

    def bcast_row(pool, dram_t, n):
        row = pool.tile([1, n], F32, tag=f"row_{dram_t.name}")
        nc.sync.dma_start(row, dram_t.ap().rearrange("n -> 1 n"))
        out = pool.tile([128, n], F32, tag=f"bc_{dram_t.name}")
        nc.gpsimd.partition_broadcast(out, row, channels=128)
        return out

    with tile.TileContext(nc) as tc, ExitStack() as top:
        pg = top.enter_context(tc.tile_pool(name="pg", bufs=1))
        dram = top.enter_context(tc.tile_pool(name="dram", bufs=1, space="DRAM"))
        pst = top.enter_context(tc.tile_pool(name="pst", bufs=4))
        pA = top.enter_context(tc.tile_pool(name="pA", bufs=1))
        pex = top.enter_context(tc.tile_pool(name="pex", bufs=6))
        p_sm = top.enter_context(tc.tile_pool(name="p_sm", bufs=2))
        p_bc = top.enter_context(tc.tile_pool(name="p_bc", bufs=2))

        ident = pg.tile([128, 128], BF16)
        make_identity(nc, ident)
        eps_t = pg.tile([128, 1], F32)
        nc.vector.memset(eps_t, EPS)
        ones8 = pg.tile([128, 2, 1], FP8)
        nc.vector.memset(ones8, 1.0)

        bq_col = pg.tile([128, 4], F32)
        nc.sync.dma_start(bq_col, bq.ap().rearrange("(m p) -> p m", p=128))
        bk_col = pg.tile([128, 4], F32)
        nc.sync.dma_start(bk_col, bk.ap().rearrange("(m p) -> p m", p=128))
        b1_col = pg.tile([128, FF // 128], F32)
        nc.sync.dma_start(b1_col, b1.ap().rearrange("(m p) -> p m", p=128))

        bv_bc = bcast_row(pg, bv, EO) if "bv" in flags else None
        bo_bc = bcast_row(pg, bo, E) if "bo" in flags else None
        b2_bc = bcast_row(pg, b2, E) if "b2" in flags else None
        g1_bc = bcast_row(pg, g1, E) if "g1" in flags else None
        be1_bc = bcast_row(pg, be1, E) if "be1" in flags else None
        g2_bc = bcast_row(pg, g2, E) if "g2" in flags else None
        be2_bc = bcast_row(pg, be2, E) if "be2" in flags else None

        bounce_ins = []
        bounce_outs = []
        for i in range(4):
            bounce_ins.append(dram.tile([512, E], BF16, tag=f"bin{i}", name=f"bin{i}"))
            bounce_outs.append(dram.tile([256, E], BF16, tag=f"bout{i}", name=f"bout{i}"))

        wq_sb = pA.tile([128, KC, EOP], FP8)
        wk_sb = pA.tile([128, KC, EOP], FP8)
        wv_sb = pA.tile([128, KC, EO], FP8)
        wo_sb = pA.tile([128, 2, 2, E], FP8)
        w1_sb = pA.tile([128, 2, EG, 2, FF], FP8)
        w2_sb = pA.tile([128, 2, FG, 2, E], FP8)
        qT = pA.tile([128, 2, 2, S], FP8)
        kT = pA.tile([128, 2, 2, S], FP8)
        vA = pA.tile([128, HPC, TBF, 80], FP8)
        aoT = pA.tile([128, 2, 2, S], FP8)
        xres_sb = pA.tile([128, TBH, E], BF16)
        x1nB = pA.tile([128, TBH, E], BF16)
        x1T = pA.tile([128, EG, 2, SH], FP8)
        hT = pA.tile([128, FG, 2, SH], FP8)

        nc.gpsimd.dma_start(wq_sb, wq.ap().rearrange("(kc p) m -> p kc m", p=128))
        nc.gpsimd.dma_start(wk_sb, wk.ap().rearrange("(kc p) m -> p kc m", p=128))
        nc.gpsimd.dma_start(wv_sb, wv.ap().rearrange("(kc p) m -> p kc m", p=128))
        nc.gpsimd.dma_start(wo_sb, wo.ap().rearrange("(g t p) e -> p g t e", p=128, t=2))
        nc.gpsimd.dma_start(
            w1_sb, w1.ap().rearrange("a (g t p) f -> p a g t f", p=128, t=2)
        )
        nc.gpsimd.dma_start(
            w2_sb, w2.ap().rearrange("a (g t p) e -> p a g t e", p=128, t=2)
        )
        nc.gpsimd.dma_start(
            xres_sb, xres.ap().rearrange("(tb p) e -> p tb e", p=128)
        )
        # out_proj reads all four (g,t) groups of aoT; (1,1) is the zero pad
        nc.vector.memset(aoT[:, 1, 1, :], 0.0)

        with ExitStack() as qkv_scope:
            pX = qkv_scope.enter_context(tc.tile_pool(name="pX", bufs=1))
            x8_sb = pX.tile([128, KC, S], FP8)
            x8_v = x8.ap().rearrange("(kc p) s -> p kc s", p=128)
            for kc in range(KC):
                nc.sync.dma_start(x8_sb[:, kc, :], x8_v[:, kc, :])

            # ---------------- QKV (fp8 DoubleRow) ----------------
            with tc.tile_pool(name="ps_qkv", bufs=3, space="PSUM") as ps_qkv:
                x8r = x8_sb.rearrange("p (g t) s -> p g t s", t=2)
                for n4 in range(4):
                    tsl = slice(n4 * 512, (n4 + 1) * 512)
                    for w_sb, dstT, bcol, hasb in (
                        (wq_sb, qT, bq_col, "bq" in flags),
                        (wk_sb, kT, bk_col, "bk" in flags),
                    ):
                        wr = w_sb.rearrange("p (g t) m -> p g t m", t=2)
                        for ch in range(4):
                            ps = ps_qkv.tile([128, 512], F32, tag="qk")
                            for g in range(EG):
                                nc.tensor.matmul(
                                    ps,
                                    wr[:, g, :, ch * 128 : (ch + 1) * 128],
                                    x8r[:, g, :, tsl],
                                    start=(g == 0), stop=(g == EG - 1),
                                    perf_mode=PM.DoubleRow,
                                )
                            dst = dstT[:, ch // 2, ch % 2, tsl]
                            if hasb:
                                nc.vector.tensor_scalar(
                                    out=dst, in0=ps, scalar1=bcol[:, ch : ch + 1],
                                    scalar2=None, op0=AOP.add,
                                )
                            elif ch % 2 == 0:
                                nc.scalar.copy(dst, ps)
                            else:
                                nc.vector.tensor_copy(dst, ps)
                    for tb in range(n4 * 4, n4 * 4 + 4):
                        ps = ps_qkv.tile([128, EO], F32, tag="v")
                        for g in range(EG):
                            nc.tensor.matmul(
                                ps,
                                x8r[:, g, :, tb * 128 : (tb + 1) * 128],
                                wv_sb.rearrange("p (g t) m -> p g t m", t=2)[:, g],
                                start=(g == 0), stop=(g == EG - 1),
                                perf_mode=PM.DoubleRow,
                            )
                        src = ps.rearrange("p (h d) -> p h d", h=HPC)
                        dst = vA[:, :, tb, 0:DH]
                        if "bv" in flags:
                            nc.vector.tensor_tensor(
                                dst, src,
                                bv_bc.rearrange("p (h d) -> p h d", h=HPC),
                                op=AOP.add,
                            )
                        elif tb % 2 == 0:
                            nc.scalar.copy(dst, src)
                        else:
                            nc.vector.tensor_copy(dst, src)

        # stage pool opens after x8 dies so the allocator can reuse its space
        p_stage = top.enter_context(tc.tile_pool(name="p_stage", bufs=2))

        # ------------- attention + out_proj + FFN (pipelined) -------------
        with tc.tile_pool(name="ps_o", bufs=2, space="PSUM") as ps_o:
            pools = {}

            def attn_block(c):
                qsl = slice(c * QB, (c + 1) * QB)
                pending = [None]

                def flush():
                    if pending[0] is not None:
                        pending[0]()
                        pending[0] = None

                for hg in range(3):
                    zc = pools['ps_z'].tile([128, 2, 2], F32, tag="zc")
                    accp = pools['ps_acc'].tile([DH, 2, QB], F32, tag="acc")
                    accs = [accp[:, 0, :], accp[:, 1, :]]
                    def emit_scores(u):
                        scs = []
                        for hi in range(2):
                            h = hg * 2 + hi
                            G, b = h // 4, h % 4
                            sc = pools['ps_sc'].tile([128, 2, QB], F32, tag="sc")
                            scs.append(sc)
                            for t in range(2):
                                kb = 2 * u + t
                                nc.tensor.matmul(
                                    sc[:, t, :],
                                    kT[32 * b : 32 * b + 32, G, :,
                                       kb * 128 : (kb + 1) * 128],
                                    qT[32 * b : 32 * b + 32, G, :, qsl],
                                    start=True, stop=True,
                                    perf_mode=PM.DoubleRow,
                                    tile_position=(32 * b, 0),
                                )
                        return scs

                    def emit_exps(scs, u):
                        exs = []
                        for hi in range(2):
                            if hi == 1 and u != 7:
                                e8 = pex.tile([128, 2, QB], I8, tag="e8")
                                nc.vector.tensor_scalar(
                                    out=e8, in0=scs[hi], scalar1=SCHRAUD_A,
                                    scalar2=SCHRAUD_B, op0=AOP.mult,
                                    op1=AOP.add,
                                )
                                exs.append(e8[:].bitcast(FP8))
                            else:
                                exf = pex.tile([128, 2, QB], FP8, tag="exf")
                                nc.scalar.activation(exf, scs[hi], ACT.Exp,
                                                     scale=SC_SCALE)
                                exs.append(exf[:])
                        return exs

                    def emit_attnv(exs, u):
                        for hi in range(2):
                            h = hg * 2 + hi
                            nc.tensor.matmul(
                                accs[hi], vA[:, h, 2 * u : 2 * u + 2, 0:DH],
                                exs[hi],
                                start=(u == 0), stop=(u == TBF // 2 - 1),
                                perf_mode=PM.DoubleRow,
                            )
                            for ch in range(2):
                                nc.tensor.matmul(
                                    zc[:, hi, ch : ch + 1],
                                    exs[hi][:, :, ch * 128 : (ch + 1) * 128],
                                    ones8[:],
                                    start=(u == 0), stop=(u == TBF // 2 - 1),
                                    perf_mode=PM.DoubleRow,
                                )

                    # software pipeline: scores(u) land in the PE queue ahead
                    # of attnv(u-1) so the PE never blocks on an in-flight exp;
                    # the pipeline carries across head-group boundaries
                    for u in range(TBF // 2):
                        scs = emit_scores(u)
                        flush()
                        exs = emit_exps(scs, u)
                        pending[0] = (
                            lambda exs=exs, u=u, f=emit_attnv: f(exs, u)
                        )
                    flush()
                    emit_norm_impl(accp, zc, hg, c, qsl)
                flush()
                return

            def emit_norm_impl(accp, zc, hg, c, qsl):
                    accs = [accp[:, 0, :], accp[:, 1, :]]
                    # recip of the head-pair's denominators (batched, free=4);
                    # transpose on the way to DRAM so each head's 256 recips
                    # are contiguous for the broadcast read
                    rc = p_sm.tile([128, 2, 2], F32, tag="rc")
                    nc.vector.reciprocal(rc, zc)
                    drc = dram.tile([2, 2, 128], F32, tag=f"drc{c}_{hg}",
                                    name=f"drc{c}_{hg}")
                    dst_ap = bass.AP(
                        tensor=drc.tensor, offset=drc.offset,
                        ap=[[1, 128], [128, 4]],
                    )
                    nc.sync.dma_start(dst_ap, rc[:])
                    for hi in range(2):
                        h = hg * 2 + hi
                        bc = p_bc.tile([DH, 2, 128], F32, tag="bc")
                        bsrc = bass.AP(
                            tensor=drc.tensor, offset=drc.offset + 256 * hi,
                            ap=[[0, DH], [1, 256]],
                        )
                        nc.sync.dma_start(bc, bsrc)
                        po = (h % 2) * DH
                        nc.vector.tensor_tensor(
                            aoT[po : po + DH, h // 4, (h // 2) % 2, qsl],
                            accs[hi].rearrange("p (a b) -> p a b", a=2), bc,
                            op=AOP.mult,
                        )

            def out_proj(tb):
                ps0 = ps_o.tile([128, 512], F32, tag="o512", name="ps0")
                ps1 = ps_o.tile([128, 512], F32, tag="o512", name="ps1")[:, 0:256]
                for g in range(2):
                    lhs = aoT[:, g, :, tb * 128 : (tb + 1) * 128]
                    nc.tensor.matmul(ps0, lhs, wo_sb[:, g, :, 0:512],
                                     start=(g == 0), stop=(g == 1),
                                     perf_mode=PM.DoubleRow)
                    nc.tensor.matmul(ps1, lhs, wo_sb[:, g, :, 512:768],
                                     start=(g == 0), stop=(g == 1),
                                     perf_mode=PM.DoubleRow)
                pos = p_stage.tile([128, E], BF16, tag="pos")
                nc.scalar.activation(pos[:, 0:512], ps0, ACT.Copy,
                                     scale=OP_SCALE)
                nc.scalar.activation(pos[:, 512:768], ps1, ACT.Copy,
                                     scale=OP_SCALE)
                nc.sync.dma_start(
                    bounce_ins[tb // 4][(tb % 4) * 128 : (tb % 4 + 1) * 128, :],
                    pos,
                )
                if not for_sim and tb % 4 == 3:
                    nc.gpsimd.collective_compute(
                        "ReduceScatter", AOP.add,
                        replica_groups=REPLICA_GROUPS,
                        ins=[bounce_ins[tb // 4][:].opt()],
                        outs=[bounce_outs[tb // 4][:].opt()],
                    )

            def ln1_block(tb):
                rs_bf = p_stage.tile([128, E], BF16, tag="rs_bf")
                nc.sync.dma_start(
                    rs_bf,
                    bounce_outs[tb // 2][(tb % 2) * 128 : (tb % 2 + 1) * 128, :],
                )
                rs = p_stage.tile([128, E], BF16, tag="rs")
                nc.gpsimd.tensor_tensor(rs, rs_bf, xres_sb[:, tb, :], op=AOP.add)
                if "bo" in flags:
                    nc.vector.tensor_tensor(rs, rs, bo_bc, op=AOP.add)
                # LN1 rstd entirely on DVE: fast-inverse-sqrt bit trick + one
                # Newton step (LN1 per-token scale error washes out via LN2)
                st = pst.tile([128, 2, 6], F32, tag="st")
                for sg in range(2):
                    nc.vector.bn_stats(st[:, sg, :], rs[:, sg * 384 : (sg + 1) * 384])
                mv = pst.tile([128, 2], F32, tag="mv")
                nc.vector.bn_aggr(mv, st)
                ri = pst.tile([128, 1], mybir.dt.int32, tag="ri")
                nc.vector.tensor_scalar(
                    out=ri, in0=mv[:, 1:2].bitcast(mybir.dt.int32),
                    scalar1=-0.5, scalar2=1597463007.0,
                    op0=AOP.mult, op1=AOP.add,
                )
                y0 = ri[:].bitcast(F32)
                t1 = pst.tile([128, 1], F32, tag="t1")
                nc.vector.tensor_tensor(t1, y0, y0, op=AOP.mult)
                t2 = pst.tile([128, 1], F32, tag="t2")
                nc.vector.tensor_tensor(t2, t1, mv[:, 1:2], op=AOP.mult)
                t3 = pst.tile([128, 1], F32, tag="t3")
                nc.vector.tensor_scalar(out=t3, in0=t2, scalar1=-0.5,
                                        scalar2=1.5, op0=AOP.mult, op1=AOP.add)
                rstd = pst.tile([128, 1], F32, tag="rstd")
                nc.vector.tensor_tensor(rstd, t3, y0, op=AOP.mult)
                mrs = pst.tile([128, 1], F32, tag="mrs")
                nc.vector.tensor_tensor(mrs, mv[:, 0:1], rstd, op=AOP.mult)
                dst = x1nB[:, tb, :]
                nc.vector.tensor_scalar(
                    out=dst, in0=rs, scalar1=rstd, scalar2=mrs,
                    op0=AOP.mult, op1=AOP.subtract,
                )
                if "g1" in flags:
                    nc.vector.tensor_tensor(dst, dst, g1_bc, op=AOP.mult)
                if "be1" in flags:
                    nc.vector.tensor_tensor(dst, dst, be1_bc, op=AOP.add)

            def fc1_block(n):
                tsl = slice(n * 512, (n + 1) * 512)
                for mf in range(FF // 128):
                    ps = ps_o.tile([128, 512], F32, tag="o512")
                    for ab in range(2):
                        for g in range(EG):
                            nc.tensor.matmul(
                                ps, w1_sb[:, ab, g, :, mf * 128 : (mf + 1) * 128],
                                x1T[:, g, :, tsl],
                                start=(ab == 0 and g == 0),
                                stop=(ab == 1 and g == EG - 1),
                                perf_mode=PM.DoubleRow,
                            )
                    nc.scalar.activation(
                        hT[:, mf // 2, mf % 2, tsl], ps, ACT.Gelu,
                        bias=b1_col[:, mf : mf + 1], scale=S64,
                    )

            def fc2_block(tb):
                ps0 = ps_o.tile([128, 512], F32, tag="o512", name="ps0")
                ps1 = ps_o.tile([128, 512], F32, tag="o512", name="ps1")[:, 0:256]
                for ab in range(2):
                    for g in range(FG):
                        lhs = hT[:, g, :, tb * 128 : (tb + 1) * 128]
                        st_, sp_ = (ab == 0 and g == 0), (ab == 1 and g == FG - 1)
                        nc.tensor.matmul(ps0, lhs, w2_sb[:, ab, g, :, 0:512],
                                         start=st_, stop=sp_,
                                         perf_mode=PM.DoubleRow)
                        nc.tensor.matmul(ps1, lhs, w2_sb[:, ab, g, :, 512:768],
                                         start=st_, stop=sp_,
                                         perf_mode=PM.DoubleRow)
                y2 = p_stage.tile([128, E], BF16, tag="y2")
                nc.vector.scalar_tensor_tensor(
                    out=y2[:, 0:512], in0=ps0, scalar=S64,
                    in1=x1nB[:, tb, 0:512], op0=AOP.mult, op1=AOP.add,
                )
                nc.vector.scalar_tensor_tensor(
                    out=y2[:, 512:768], in0=ps1, scalar=S64,
                    in1=x1nB[:, tb, 512:768], op0=AOP.mult, op1=AOP.add,
                )
                if "b2" in flags:
                    nc.vector.tensor_tensor(y2, y2, b2_bc, op=AOP.add)
                # LN2 (exact sqrt path; runs after attention so one table load)
                st = pst.tile([128, 2, 6], F32, tag="st")
                for sg in range(2):
                    nc.vector.bn_stats(st[:, sg, :], y2[:, sg * 384 : (sg + 1) * 384])
                mv = pst.tile([128, 2], F32, tag="mv")
                nc.vector.bn_aggr(mv, st)
                sv = pst.tile([128, 1], F32, tag="sv")
                nc.scalar.activation(sv, mv[:, 1:2], ACT.Sqrt, bias=eps_t[:, 0:1])
                rstd = pst.tile([128, 1], F32, tag="rstd")
                nc.vector.reciprocal(rstd, sv)
                mrs = pst.tile([128, 1], F32, tag="mrs")
                nc.vector.tensor_tensor(mrs, mv[:, 0:1], rstd, op=AOP.mult)
                yt = p_stage.tile([128, E], F32, tag="yt")
                nc.vector.tensor_scalar(
                    out=yt, in0=y2, scalar1=rstd, scalar2=mrs,
                    op0=AOP.mult, op1=AOP.subtract,
                )
                if "g2" in flags:
                    nc.vector.tensor_tensor(yt, yt, g2_bc, op=AOP.mult)
                if "be2" in flags:
                    nc.vector.tensor_tensor(yt, yt, be2_bc, op=AOP.add)
                nc.sync.dma_start(y[tb * 128 : (tb + 1) * 128, :], yt)

            # fc1/fc2 (and their gelus) are emitted after the attention loop:
            # the Tile scheduler otherwise interleaves ready gelus 1:1 with
            # exps and every pair costs two activation-table loads.
            with (
                tc.tile_pool(name="ps_sc", bufs=3, space="PSUM") as ps_sc_,
                tc.tile_pool(name="ps_acc", bufs=2, space="PSUM") as ps_acc_,
                tc.tile_pool(name="ps_z", bufs=1, space="PSUM") as ps_z_,
            ):
                pools.update(ps_sc=ps_sc_, ps_acc=ps_acc_, ps_z=ps_z_)
                # out_proj for block c is emitted one attention block late
                # so its PE matmuls never wait on the recip->broadcast->
                # normalize chain of their own block
                for c in range(NC):
                    attn_block(c)
                    if c > 0:
                        out_proj(2 * c - 2)
                        out_proj(2 * c - 1)
                    if c == 3:
                        ln1_block(0)
                        ln1_block(1)
                    if c == 5:
                        ln1_block(2)
                        ln1_block(3)
                    if c == 7:
                        ln1_block(4)
                        ln1_block(5)
                out_proj(2 * NC - 2)
                out_proj(2 * NC - 1)
                ln1_block(6)
                ln1_block(7)
            with tc.tile_pool(name="ps_pt", bufs=2, space="PSUM") as ps_pt:
                def transpose_block(tb):
                    for eg in range(EG):
                        pt = ps_pt.tile([128, 2, 128], BF16, tag="pt")
                        for t in range(2):
                            ec = eg * 2 + t
                            nc.tensor.transpose(
                                pt[:, t, :],
                                x1nB[:, tb, ec * 128 : (ec + 1) * 128], ident,
                            )
                        nc.vector.tensor_copy(
                            x1T[:, eg, :, tb * 128 : (tb + 1) * 128], pt
                        )

                for tb in range(4):
                    transpose_block(tb)
                fc1_block(0)
                for tb in range(4, TBH):
                    transpose_block(tb)
                fc2_block(0)
                fc2_block(1)
                fc1_block(1)
                fc2_block(2)
                fc2_block(3)
                for tb in range(4, TBH):
                    fc2_block(tb)

    nc.compile()
    return nc


_PROGRAM_CACHE = {}


def _get_program(flags):
    key = frozenset(flags)
    if key not in _PROGRAM_CACHE:
        _PROGRAM_CACHE[key] = build_program(key)
    return _PROGRAM_CACHE[key]


def _prep_inputs(inputs):
    f32 = lambda a: np.ascontiguousarray(np.asarray(a, dtype=np.float32))
    fp8 = lambda a: np.ascontiguousarray(np.asarray(a, dtype=np.float32)).astype(NPF8)
    bf = lambda a: np.ascontiguousarray(np.asarray(a, dtype=np.float32)).astype(NPBF)

    x = f32(inputs["x"])
    Wq, Wk, Wv, Wo = (f32(inputs[k]) for k in ("Wq", "Wk", "Wv", "Wo"))
    W1, W2 = f32(inputs["W1"]), f32(inputs["W2"])
    bq_, bk_, bv_, bo_ = (f32(inputs[k]) for k in ("bq", "bk", "bv", "bo"))
    b1_, b2_ = f32(inputs["b1"]), f32(inputs["b2"])
    g1_, be1_ = f32(inputs["ln1_g"]), f32(inputs["ln1_b"])
    g2_, be2_ = f32(inputs["ln2_g"]), f32(inputs["ln2_b"])

    scaling = DH ** -0.5
    flags = set()
    for name, arr in (("bq", bq_), ("bk", bk_), ("bv", bv_), ("bo", bo_),
                      ("b2", b2_), ("be1", be1_), ("be2", be2_)):
        if np.any(arr):
            flags.add(name)
    if np.any(g1_ != 1.0):
        flags.add("g1")
    if np.any(g2_ != 1.0):
        flags.add("g2")

    perm = np.zeros(EOP, dtype=np.int64)
    valid = np.zeros(EOP, dtype=bool)
    for cidx in range(EOP):
        chunk, p = cidx // 128, cidx % 128
        G, T = chunk // 2, chunk % 2
        head = G * 4 + p // 32
        d = T * 32 + p % 32
        if head < HPC:
            perm[cidx] = head * DH + d
            valid[cidx] = True

    def feedback(Wm):
        a = (Wm * W12_SC).astype(NPF8)
        b_ = (Wm * W12_SC - a.astype(np.float32)).astype(NPF8)
        return np.stack([a, b_], axis=0)

    in_maps = []
    for c in range(NCORES):
        b, j = divmod(c, 2)
        xb = x[:, b, :]
        sl = slice(j * EO, (j + 1) * EO)
        rows = [slice(512 * q + 256 * j, 512 * q + 256 * j + 256) for q in range(4)]

        wq_sl = Wq[:, sl] * (scaling * QK_SC)
        wk_sl = Wk[:, sl] * QK_SC
        wq_p = np.zeros((E, EOP), np.float32)
        wk_p = np.zeros((E, EOP), np.float32)
        wq_p[:, valid] = wq_sl[:, perm[valid]]
        wk_p[:, valid] = wk_sl[:, perm[valid]]
        bq_p = np.zeros(EOP, np.float32)
        bk_p = np.zeros(EOP, np.float32)
        bq_p[valid] = bq_[sl][perm[valid]] * (scaling * QK_SC)
        bk_p[valid] = bk_[sl][perm[valid]] * QK_SC

        wo_p = np.zeros((EOP, E), np.float32)
        wo_p[:EO] = Wo[sl, :] * WO_SC

        m = {
            "x8": fp8(xb.T),
            "xres": bf(np.concatenate([xb[r] for r in rows], axis=0)),
            "wq": fp8(wq_p),
            "wk": fp8(wk_p),
            "wv": fp8(Wv[:, sl] * V_SC),
            "wo": fp8(wo_p),
            "w1": np.ascontiguousarray(feedback(W1)),
            "w2": np.ascontiguousarray(feedback(W2)),
            "bq": f32(bq_p),
            "bk": f32(bk_p),
            "bv": f32(bv_[sl] * V_SC),
            "bo": f32(bo_),
            "b1": f32(b1_),
            "b2": f32(b2_),
            "g1": f32(g1_),
            "be1": f32(be1_),
            "g2": f32(g2_),
            "be2": f32(be2_),
        }
        in_maps.append(m)
    return in_maps, flags


def run(inputs, **spmd_kwargs):
    in_maps, flags = _prep_inputs(inputs)
    nc = _get_program(flags)
    try:
        res = run_bass_kernel_spmd(
            nc, in_maps, core_ids=list(range(NCORES)), **spmd_kwargs
        )
    except Exception:
        res = run_bass_kernel_spmd(
            nc, in_maps, core_ids=list(range(NCORES)), **spmd_kwargs
        )
    out = np.empty((S, B, E), dtype=np.float32)
    for c in range(NCORES):
        b, j = divmod(c, 2)
        yc = res.results[c]["y"]
        for q in range(4):
            r = slice(512 * q + 256 * j, 512 * q + 256 * j + 256)
            out[r, b, :] = yc[256 * q : 256 * q + 256]
    return out, res


def kernel(**inputs):
    out, _ = run(inputs)
    return out
